# revision 6
# baseline (speedup 1.0000x reference)
"""3-layer GCN on 8 Trainium2 NeuronCores (Bass/Tile).

Math (per layer, identical to PyG GCNConv with self-loops):
    x_{l+1} = A_hat @ (x_l @ W_l) + b_l,   A_hat = D^-1/2 (A+I) D^-1/2

We use associativity to aggregate FIRST and GEMM second:
    x_{l+1} = (A_hat @ x_l) @ W_l + b_l
so each core only runs the 128x128 GEMM on its own 1/8 of the nodes.

Sharding: node v -> core (v % 8), local row j = v // 8.  Each core owns
aggregation + GEMM for its 12500 destination rows.  Between layers an
AllGather over internal DRAM rebuilds the full (permuted) feature table
x_perm[(v%8)*12500 + v//8] = x[v] that the next layer's gathers read.

Per-edge work on device (per destination tile of 128 nodes):
  - indirect_dma_start (dynamic-offset DGE) pulls the 512B source rows
    for 128 edges per "chunk" into SBUF, one edge per partition, with
    int32 row offsets host-precomputed per chunk.
  - VectorE builds S^T[e,d] = (iota[d] == dst_local[e]) * norm[e] in one
    tensor_scalar op per chunk.
  - TensorE accumulates aggT[f,d] += msg_chunk^T @ S^T in PSUM.
  - aggT feeds the GEMM directly as lhsT; bias is added with a rank-1
    (ones x bias) accumulating matmul; ScalarE copies PSUM->SBUF.

All edge metadata (int32 gather rows, dst_local and norm per edge slot)
is host-precomputed, padded to 128-edge chunks (pad: row=0, norm=0) and
streamed per tile-group.
"""

import numpy as np

# ----------------------------------------------------------------- config

FULL_CFG = dict(
    N=100000,          # nodes
    D=128,             # feature dim (= hidden)
    CORES=8,
    SLOT_BUDGET=144,   # max message chunks resident per tile-group
)


def _derive(cfg):
    c = dict(cfg)
    c["NPC"] = c["N"] // c["CORES"]            # nodes per core
    assert c["NPC"] * c["CORES"] == c["N"]
    c["NT"] = (c["NPC"] + 127) // 128          # dst tiles per core
    return c


# ----------------------------------------------------- host preprocessing

def _edge_layout(cfg, dst_core, dst_local):
    """Chunk layout (identical across cores, SPMD program)."""
    CORES, NT = cfg["CORES"], cfg["NT"]
    tile = dst_local // 128
    key = (dst_core * NT + tile).astype(np.int64)
    counts = np.bincount(key, minlength=CORES * NT).reshape(CORES, NT)
    ct = (counts.max(axis=0) + 127) // 128             # chunks per tile [NT]
    budget = cfg["SLOT_BUDGET"]
    assert ct.max() <= budget
    groups, cur, cur_s = [], [], 0
    for t in range(NT):
        s = int(ct[t])
        if cur_s + s > budget:
            groups.append(cur)
            cur, cur_s = [], 0
        cur.append(t)
        cur_s += s
    if cur:
        groups.append(cur)

    ginfo, goff = [], 0
    slot_base = np.zeros(NT, np.int64)     # global slot of tile start
    for g, tiles in enumerate(groups):
        S_g = int(sum(ct[t] for t in tiles))
        off, tile_slots = 0, {}
        for t in tiles:
            slot_base[t] = goff + off
            tile_slots[t] = (off, int(ct[t]))    # local slot range in group
            off += int(ct[t])
        ginfo.append(dict(tiles=tiles, S_g=S_g, goff=goff,
                          tile_slots=tile_slots))
        goff += S_g
    return dict(ct=ct, groups=groups, ginfo=ginfo, tot_slots=goff,
                key=key, tile=tile, slot_base=slot_base)


def _fill_blobs(cfg, lay, src_rows, dst_core, dst_local, norm):
    """Per-core idx/dst/norm blobs [CORES, 128, tot_slots]."""
    CORES, NT = cfg["CORES"], cfg["NT"]
    tot = lay["tot_slots"]
    key, tile, slot_base = lay["key"], lay["tile"], lay["slot_base"]

    order = np.argsort(key, kind="stable")
    counts_flat = np.bincount(key, minlength=CORES * NT)
    seg_off = np.concatenate([[0], np.cumsum(counts_flat)])
    rank_sorted = np.arange(len(order)) - seg_off[key[order]]
    rank = np.empty(len(order), np.int64)
    rank[order] = rank_sorted

    gslot = slot_base[tile] + rank // 128
    part = rank % 128

    idx_blob = np.zeros((CORES, 128, tot), np.int32)
    meta_dst = np.zeros((CORES, 128, tot), np.float32)
    meta_nrm = np.zeros((CORES, 128, tot), np.float32)
    idx_blob[dst_core, part, gslot] = src_rows.astype(np.int32)
    meta_dst[dst_core, part, gslot] = (dst_local - tile * 128).astype(
        np.float32)
    meta_nrm[dst_core, part, gslot] = norm.astype(np.float32)
    return idx_blob, meta_dst, meta_nrm


def preprocess(cfg, edge_index):
    N, CORES, NPC = cfg["N"], cfg["CORES"], cfg["NPC"]
    ei = np.asarray(edge_index).astype(np.int64)
    src = np.concatenate([ei[0], np.arange(N, dtype=np.int64)])
    dst = np.concatenate([ei[1], np.arange(N, dtype=np.int64)])
    deg = np.bincount(dst, minlength=N).astype(np.float32)
    dinv = (1.0 / np.sqrt(deg)).astype(np.float32)
    norm = dinv[src] * dinv[dst]

    dst_core = (dst % CORES).astype(np.int64)
    dst_local = (dst // CORES).astype(np.int64)

    lay = _edge_layout(cfg, dst_core, dst_local)
    # layer 0 gathers from node_features (original numbering);
    # layers 1,2 gather from the AllGather output (permuted numbering).
    blobs0 = _fill_blobs(cfg, lay, src, dst_core, dst_local, norm)
    prow = (src % CORES) * NPC + src // CORES
    blobs12 = _fill_blobs(cfg, lay, prow, dst_core, dst_local, norm)
    return lay, blobs0, blobs12


# -------------------------------------------------------- device program

def build_program(cfg, lay, n_layers=3, use_collective=True):
    import concourse.bass as bass
    import concourse.bacc as bacc
    import concourse.tile as tile
    import concourse.mybir as mybir

    f32 = mybir.dt.float32
    i32 = mybir.dt.int32
    N, D, CORES = cfg["N"], cfg["D"], cfg["CORES"]
    NPC, NT = cfg["NPC"], cfg["NT"]
    BUD = cfg["SLOT_BUDGET"]
    tot = lay["tot_slots"]

    nc = bacc.Bacc("TRN2", target_bir_lowering=False, debug=False,
                   num_devices=CORES)

    x0 = nc.dram_tensor("node_features", [N, D], f32, kind="ExternalInput")
    iota_in = nc.dram_tensor("iota", [128, 128], f32, kind="ExternalInput")
    W_in = [nc.dram_tensor(f"W{l}", [D, D], f32, kind="ExternalInput")
            for l in range(3)]
    B_in = [nc.dram_tensor(f"b{l}", [1, D], f32, kind="ExternalInput")
            for l in range(3)]
    idx_in = [nc.dram_tensor(f"idxL{i}", [128, tot], i32,
                             kind="ExternalInput") for i in range(2)]
    dst_in = nc.dram_tensor("dstL", [128, tot], f32, kind="ExternalInput")
    nrm_in = nc.dram_tensor("nrmL", [128, tot], f32, kind="ExternalInput")
    y_out = nc.dram_tensor("y_out", [NPC, D], f32, kind="ExternalOutput")

    xs = [nc.dram_tensor(f"xslice{l}", [NPC, D], f32) for l in range(2)]
    xg = [nc.dram_tensor(f"xgath{l}", [N, D], f32, addr_space="Shared")
          for l in range(2)]

    with tile.TileContext(nc) as tc:
        with (
            tc.tile_pool(name="const", bufs=1) as constp,
            tc.tile_pool(name="msg", bufs=2) as msgp,
            tc.tile_pool(name="meta", bufs=3) as metap,
            tc.tile_pool(name="st", bufs=6) as stp,
            tc.tile_pool(name="sb2", bufs=3) as sb2p,
            tc.tile_pool(name="psum", bufs=2, space="PSUM") as psump,
        ):
            iota_sb = constp.tile([128, 128], f32, tag="iota")
            nc.sync.dma_start(iota_sb[:], iota_in[:, :])
            ones_sb = constp.tile([1, 128], f32, tag="ones")
            nc.vector.memset(ones_sb[:], 1.0)
            W_sb, B_sb = [], []
            for l in range(3):
                w = constp.tile([128, 128], f32, tag=f"W{l}")
                nc.sync.dma_start(w[:], W_in[l][:, :])
                W_sb.append(w)
                b = constp.tile([1, 128], f32, tag=f"b{l}")
                nc.sync.dma_start(b[:], B_in[l][:, :])
                B_sb.append(b)

            for l in range(n_layers):
                li = 0 if l == 0 else 1
                x_src = x0 if l == 0 else xg[l - 1]
                tgt = y_out if l == n_layers - 1 else xs[l]
                for gi in lay["ginfo"]:
                    S_g = gi["S_g"]
                    goff = gi["goff"]
                    msg = msgp.tile([128, BUD * 128], f32, tag="msg")
                    msg3 = msg[:].rearrange("p (s e) -> p s e", e=128)
                    idxt = metap.tile([128, BUD], i32, tag="idx")
                    dstt = metap.tile([128, BUD], f32, tag="dst")
                    nrmt = metap.tile([128, BUD], f32, tag="nrm")
                    nc.sync.dma_start(idxt[:, :S_g],
                                      idx_in[li][:, goff:goff + S_g])
                    nc.sync.dma_start(dstt[:, :S_g],
                                      dst_in[:, goff:goff + S_g])
                    nc.sync.dma_start(nrmt[:, :S_g],
                                      nrm_in[:, goff:goff + S_g])
                    for t in gi["tiles"]:
                        off, n_ch = gi["tile_slots"][t]
                        psA = psump.tile([128, 128], f32, tag="psA")
                        for j in range(n_ch):
                            s = off + j
                            nc.gpsimd.indirect_dma_start(
                                out=msg3[:, s, :],
                                out_offset=None,
                                in_=x_src[:, :],
                                in_offset=bass.IndirectOffsetOnAxis(
                                    ap=idxt[:, s:s + 1], axis=0),
                            )
                            stt = stp.tile([128, 128], f32, tag="st")
                            nc.vector.tensor_scalar(
                                stt[:], iota_sb[:],
                                dstt[:, s:s + 1], nrmt[:, s:s + 1],
                                mybir.AluOpType.is_equal,
                                mybir.AluOpType.mult,
                            )
                            nc.tensor.matmul(
                                psA[:], msg3[:, s, :], stt[:],
                                start=(j == 0), stop=(j == n_ch - 1),
                            )
                        aggT = sb2p.tile([128, 128], f32, tag="aggT")
                        nc.scalar.copy(aggT[:], psA[:])
                        psY = psump.tile([128, 128], f32, tag="psY")
                        nc.tensor.matmul(psY[:], aggT[:], W_sb[l][:],
                                         start=True, stop=False)
                        nc.tensor.matmul(psY[:], ones_sb[:], B_sb[l][:],
                                         start=False, stop=True)
                        ysb = sb2p.tile([128, 128], f32, tag="ysb")
                        nc.scalar.copy(ysb[:], psY[:])
                        rows = min(128, NPC - t * 128)
                        nc.sync.dma_start(tgt[t * 128:t * 128 + rows, :],
                                          ysb[:rows, :])
                if l < n_layers - 1 and use_collective:
                    nc.gpsimd.collective_compute(
                        "AllGather",
                        mybir.AluOpType.bypass,
                        replica_groups=[list(range(CORES))],
                        ins=[xs[l][:, :].opt()],
                        outs=[xg[l][:, :].opt()],
                    )
    nc.compile()
    return nc


# ------------------------------------------------------------- execution

def make_in_maps(cfg, inputs, lay, blobs0, blobs12):
    CORES, D = cfg["CORES"], cfg["D"]
    idx0, dst0, nrm0 = blobs0
    idx12, _, _ = blobs12
    iota = np.tile(np.arange(128, dtype=np.float32), (128, 1))
    nf = np.ascontiguousarray(np.asarray(inputs["node_features"],
                                         dtype=np.float32))
    in_maps = []
    for c in range(CORES):
        m = {
            "node_features": nf,
            "iota": iota,
            "idxL0": np.ascontiguousarray(idx0[c]),
            "idxL1": np.ascontiguousarray(idx12[c]),
            "dstL": np.ascontiguousarray(dst0[c]),
            "nrmL": np.ascontiguousarray(nrm0[c]),
        }
        for l in range(3):
            m[f"W{l}"] = np.ascontiguousarray(
                np.asarray(inputs[f"W{l}"], dtype=np.float32))
            m[f"b{l}"] = np.ascontiguousarray(
                np.asarray(inputs[f"b{l}"], dtype=np.float32).reshape(1, D))
        in_maps.append(m)
    return in_maps


def unshard_output(cfg, results):
    N, D, CORES, NPC = cfg["N"], cfg["D"], cfg["CORES"], cfg["NPC"]
    out = np.empty((N, D), np.float32)
    for c in range(CORES):
        out[c::CORES] = results[c]["y_out"][:NPC]
    return out


def kernel(**inputs) -> np.ndarray:
    cfg = _derive(FULL_CFG)
    lay, blobs0, blobs12 = preprocess(cfg, inputs["edge_index"])
    nc = build_program(cfg, lay)
    in_maps = make_in_maps(cfg, inputs, lay, blobs0, blobs12)
    from concourse import bass_utils
    res = bass_utils.run_bass_kernel_spmd(
        nc, in_maps, core_ids=list(range(cfg["CORES"])))
    return unshard_output(cfg, res.results)


# revision 10
# speedup vs baseline: 7.4519x; 7.4519x over previous
"""3-layer GCN on 8 Trainium2 NeuronCores (Bass/Tile).

Math (per layer, identical to PyG GCNConv with self-loops):
    x_{l+1} = A_hat @ (x_l @ W_l) + b_l,   A_hat = D^-1/2 (A+I) D^-1/2

We use associativity to aggregate FIRST and GEMM second:
    x_{l+1} = (A_hat @ x_l) @ W_l + b_l
so each core only runs the 128x128 GEMM on its own 1/8 of the nodes.

Sharding: node v -> core (v % 8), local row j = v // 8.  Each core owns
aggregation + GEMM for its 12500 destination rows.  Between layers an
AllGather over internal DRAM rebuilds the full (permuted) feature table
x_perm[(v%8)*12500 + v//8] = x[v] that the next layer's gathers read.

Per-edge work on device (per destination tile of 128 nodes):
  - indirect_dma_start (dynamic-offset DGE) pulls the 512B source rows
    for 128 edges per "chunk" into SBUF, one edge per partition, with
    int32 row offsets host-precomputed per chunk.
  - VectorE builds S^T[e,d] = (iota[d] == dst_local[e]) * norm[e] in one
    tensor_scalar op per chunk.
  - TensorE accumulates aggT[f,d] += msg_chunk^T @ S^T in PSUM.
  - aggT feeds the GEMM directly as lhsT; bias is added with a rank-1
    (ones x bias) accumulating matmul; ScalarE copies PSUM->SBUF.

All edge metadata (int32 gather rows, dst_local and norm per edge slot)
is host-precomputed, padded to 128-edge chunks (pad: row=0, norm=0) and
streamed per tile-group.
"""

import numpy as np

# ----------------------------------------------------------------- config

FULL_CFG = dict(
    N=100000,          # nodes
    D=128,             # feature dim (= hidden)
    CORES=8,
    SLOT_BUDGET=144,   # max message chunks resident per tile-group
)


def _derive(cfg):
    c = dict(cfg)
    c["NPC"] = c["N"] // c["CORES"]            # nodes per core
    assert c["NPC"] * c["CORES"] == c["N"]
    c["NT"] = (c["NPC"] + 127) // 128          # dst tiles per core
    return c


# ----------------------------------------------------- host preprocessing

def _edge_layout(cfg, dst_core, dst_local):
    """Chunk layout (identical across cores, SPMD program)."""
    CORES, NT = cfg["CORES"], cfg["NT"]
    tile = dst_local // 128
    key = (dst_core * NT + tile).astype(np.int64)
    counts = np.bincount(key, minlength=CORES * NT).reshape(CORES, NT)
    ct = (counts.max(axis=0) + 127) // 128             # chunks per tile [NT]
    budget = cfg["SLOT_BUDGET"]
    assert ct.max() <= budget
    groups, cur, cur_s = [], [], 0
    for t in range(NT):
        s = int(ct[t])
        if cur_s + s > budget:
            groups.append(cur)
            cur, cur_s = [], 0
        cur.append(t)
        cur_s += s
    if cur:
        groups.append(cur)

    ginfo, goff = [], 0
    slot_base = np.zeros(NT, np.int64)     # global slot of tile start
    for g, tiles in enumerate(groups):
        S_g = int(sum(ct[t] for t in tiles))
        off, tile_slots = 0, {}
        for t in tiles:
            slot_base[t] = goff + off
            tile_slots[t] = (off, int(ct[t]))    # local slot range in group
            off += int(ct[t])
        ginfo.append(dict(tiles=tiles, S_g=S_g, goff=goff,
                          tile_slots=tile_slots))
        goff += S_g
    return dict(ct=ct, groups=groups, ginfo=ginfo, tot_slots=goff,
                key=key, tile=tile, slot_base=slot_base)


def _fill_blobs(cfg, lay, src_rows, dst_core, dst_local, norm):
    """Per-core idx/dst/norm blobs [CORES, 128, tot_slots]."""
    CORES, NT = cfg["CORES"], cfg["NT"]
    tot = lay["tot_slots"]
    key, tile, slot_base = lay["key"], lay["tile"], lay["slot_base"]

    order = np.argsort(key, kind="stable")
    counts_flat = np.bincount(key, minlength=CORES * NT)
    seg_off = np.concatenate([[0], np.cumsum(counts_flat)])
    rank_sorted = np.arange(len(order)) - seg_off[key[order]]
    rank = np.empty(len(order), np.int64)
    rank[order] = rank_sorted

    gslot = slot_base[tile] + rank // 128
    part = rank % 128

    idx_blob = np.zeros((CORES, 128, tot), np.int32)
    meta_dst = np.zeros((CORES, 128, tot), np.float32)
    meta_nrm = np.zeros((CORES, 128, tot), np.float32)
    idx_blob[dst_core, part, gslot] = src_rows.astype(np.int32)
    meta_dst[dst_core, part, gslot] = (dst_local - tile * 128).astype(
        np.float32)
    meta_nrm[dst_core, part, gslot] = norm.astype(np.float32)
    return idx_blob, meta_dst, meta_nrm


def preprocess(cfg, edge_index):
    N, CORES, NPC = cfg["N"], cfg["CORES"], cfg["NPC"]
    ei = np.asarray(edge_index).astype(np.int64)
    src = np.concatenate([ei[0], np.arange(N, dtype=np.int64)])
    dst = np.concatenate([ei[1], np.arange(N, dtype=np.int64)])
    deg = np.bincount(dst, minlength=N).astype(np.float32)
    dinv = (1.0 / np.sqrt(deg)).astype(np.float32)
    norm = dinv[src] * dinv[dst]

    dst_core = (dst % CORES).astype(np.int64)
    dst_local = (dst // CORES).astype(np.int64)

    lay = _edge_layout(cfg, dst_core, dst_local)
    # layer 0 gathers from node_features (original numbering);
    # layers 1,2 gather from the AllGather output (permuted numbering).
    blobs0 = _fill_blobs(cfg, lay, src, dst_core, dst_local, norm)
    prow = (src % CORES) * NPC + src // CORES
    blobs12 = _fill_blobs(cfg, lay, prow, dst_core, dst_local, norm)
    return lay, blobs0, blobs12


# -------------------------------------------------------- device program

def build_program(cfg, lay, n_layers=3, use_collective=True):
    import concourse.bass as bass
    import concourse.bacc as bacc
    import concourse.tile as tile
    import concourse.mybir as mybir

    f32 = mybir.dt.float32
    i32 = mybir.dt.int32
    N, D, CORES = cfg["N"], cfg["D"], cfg["CORES"]
    NPC, NT = cfg["NPC"], cfg["NT"]
    BUD = cfg["SLOT_BUDGET"]
    tot = lay["tot_slots"]

    nc = bacc.Bacc("TRN2", target_bir_lowering=False, debug=False,
                   num_devices=CORES)

    x0 = nc.dram_tensor("node_features", [N, D], f32, kind="ExternalInput")
    iota_in = nc.dram_tensor("iota", [128, 128], f32, kind="ExternalInput")
    W_in = [nc.dram_tensor(f"W{l}", [D, D], f32, kind="ExternalInput")
            for l in range(3)]
    B_in = [nc.dram_tensor(f"b{l}", [1, D], f32, kind="ExternalInput")
            for l in range(3)]
    idx_in = [nc.dram_tensor(f"idxL{i}", [128, tot], i32,
                             kind="ExternalInput") for i in range(2)]
    dst_in = nc.dram_tensor("dstL", [128, tot], f32, kind="ExternalInput")
    nrm_in = nc.dram_tensor("nrmL", [128, tot], f32, kind="ExternalInput")
    y_out = nc.dram_tensor("y_out", [NPC, D], f32, kind="ExternalOutput")

    xs = [nc.dram_tensor(f"xslice{l}", [NPC, D], f32) for l in range(2)]
    xg = [nc.dram_tensor(f"xgath{l}", [N, D], f32, addr_space="Shared")
          for l in range(2)]

    with tile.TileContext(nc) as tc:
        with (
            tc.tile_pool(name="const", bufs=1) as constp,
            tc.tile_pool(name="msg", bufs=2) as msgp,
            tc.tile_pool(name="meta", bufs=3) as metap,
            tc.tile_pool(name="st", bufs=6) as stp,
            tc.tile_pool(name="sb2", bufs=3) as sb2p,
            tc.tile_pool(name="psum", bufs=2, space="PSUM") as psump,
        ):
            iota_sb = constp.tile([128, 128], f32, tag="iota")
            nc.sync.dma_start(iota_sb[:], iota_in[:, :])
            ones_sb = constp.tile([1, 128], f32, tag="ones")
            nc.vector.memset(ones_sb[:], 1.0)
            W_sb, B_sb = [], []
            for l in range(3):
                w = constp.tile([128, 128], f32, tag=f"W{l}")
                nc.sync.dma_start(w[:], W_in[l][:, :])
                W_sb.append(w)
                b = constp.tile([1, 128], f32, tag=f"b{l}")
                nc.sync.dma_start(b[:], B_in[l][:, :])
                B_sb.append(b)

            for l in range(n_layers):
                li = 0 if l == 0 else 1
                x_src = x0 if l == 0 else xg[l - 1]
                tgt = y_out if l == n_layers - 1 else xs[l]
                for gi in lay["ginfo"]:
                    S_g = gi["S_g"]
                    goff = gi["goff"]
                    msg = msgp.tile([128, BUD * 128], f32, tag="msg")
                    msg3 = msg[:].rearrange("p (s e) -> p s e", e=128)
                    idxt = metap.tile([128, BUD], i32, tag="idx")
                    dstt = metap.tile([128, BUD], f32, tag="dst")
                    nrmt = metap.tile([128, BUD], f32, tag="nrm")
                    nc.sync.dma_start(idxt[:, :S_g],
                                      idx_in[li][:, goff:goff + S_g])
                    nc.sync.dma_start(dstt[:, :S_g],
                                      dst_in[:, goff:goff + S_g])
                    nc.sync.dma_start(nrmt[:, :S_g],
                                      nrm_in[:, goff:goff + S_g])
                    for t in gi["tiles"]:
                        off, n_ch = gi["tile_slots"][t]
                        psA = psump.tile([128, 128], f32, tag="psA")
                        # one indirect DMA per 128-edge chunk: the runtime
                        # supports exactly one dynamic row offset per
                        # partition per instruction (multi-offset APs
                        # silently stream consecutive rows instead).
                        for j in range(n_ch):
                            s = off + j
                            nc.gpsimd.indirect_dma_start(
                                out=msg3[:, s, :],
                                out_offset=None,
                                in_=x_src[:, :],
                                in_offset=bass.IndirectOffsetOnAxis(
                                    ap=idxt[:, s:s + 1], axis=0),
                            )
                            stt = stp.tile([128, 128], f32, tag="st")
                            nc.vector.tensor_scalar(
                                stt[:], iota_sb[:],
                                dstt[:, s:s + 1], nrmt[:, s:s + 1],
                                mybir.AluOpType.is_equal,
                                mybir.AluOpType.mult,
                            )
                            nc.tensor.matmul(
                                psA[:], msg3[:, s, :], stt[:],
                                start=(j == 0), stop=(j == n_ch - 1),
                            )
                        aggT = sb2p.tile([128, 128], f32, tag="aggT")
                        nc.scalar.copy(aggT[:], psA[:])
                        psY = psump.tile([128, 128], f32, tag="psY")
                        nc.tensor.matmul(psY[:], aggT[:], W_sb[l][:],
                                         start=True, stop=False)
                        nc.tensor.matmul(psY[:], ones_sb[:], B_sb[l][:],
                                         start=False, stop=True)
                        ysb = sb2p.tile([128, 128], f32, tag="ysb")
                        nc.scalar.copy(ysb[:], psY[:])
                        rows = min(128, NPC - t * 128)
                        nc.sync.dma_start(tgt[t * 128:t * 128 + rows, :],
                                          ysb[:rows, :])
                if l < n_layers - 1 and use_collective:
                    nc.gpsimd.collective_compute(
                        "AllGather",
                        mybir.AluOpType.bypass,
                        replica_groups=[list(range(CORES))],
                        ins=[xs[l][:, :].opt()],
                        outs=[xg[l][:, :].opt()],
                    )
    nc.compile()
    return nc


# ------------------------------------------------------------- execution

def make_in_maps(cfg, inputs, lay, blobs0, blobs12):
    CORES, D = cfg["CORES"], cfg["D"]
    idx0, dst0, nrm0 = blobs0
    idx12, _, _ = blobs12
    iota = np.tile(np.arange(128, dtype=np.float32), (128, 1))
    nf = np.ascontiguousarray(np.asarray(inputs["node_features"],
                                         dtype=np.float32))
    in_maps = []
    for c in range(CORES):
        m = {
            "node_features": nf,
            "iota": iota,
            "idxL0": np.ascontiguousarray(idx0[c]),
            "idxL1": np.ascontiguousarray(idx12[c]),
            "dstL": np.ascontiguousarray(dst0[c]),
            "nrmL": np.ascontiguousarray(nrm0[c]),
        }
        for l in range(3):
            m[f"W{l}"] = np.ascontiguousarray(
                np.asarray(inputs[f"W{l}"], dtype=np.float32))
            m[f"b{l}"] = np.ascontiguousarray(
                np.asarray(inputs[f"b{l}"], dtype=np.float32).reshape(1, D))
        in_maps.append(m)
    return in_maps


def unshard_output(cfg, results):
    N, D, CORES, NPC = cfg["N"], cfg["D"], cfg["CORES"], cfg["NPC"]
    out = np.empty((N, D), np.float32)
    for c in range(CORES):
        out[c::CORES] = results[c]["y_out"][:NPC]
    return out


def kernel(**inputs) -> np.ndarray:
    cfg = _derive(FULL_CFG)
    lay, blobs0, blobs12 = preprocess(cfg, inputs["edge_index"])
    nc = build_program(cfg, lay)
    in_maps = make_in_maps(cfg, inputs, lay, blobs0, blobs12)
    from concourse import bass_utils
    res = bass_utils.run_bass_kernel_spmd(
        nc, in_maps, core_ids=list(range(cfg["CORES"])))
    return unshard_output(cfg, res.results)
